# revision 14
# baseline (speedup 1.0000x reference)
"""Trainium2 Bass kernel for nn_Attention_21895743275585.

Reference computation (per batch b of 4):
  qkv = w_qkv @ x_flat            # 1x1 conv, x_flat [C=256, N=2304]
  q,k l2-normalized per (head, n) along dim_head=64; SCALE=10
  sim = 10 * qhat^T khat per head; attn = softmax(sim, axis=-1)
  out = attn @ v; final = w_out @ out_inner + b_out

Sharding: 8 cores = (batch b, head-half). Each core handles 4 of the 8 heads
of one batch; host sums the partial output projections and adds b_out.

Schedule (v3): the kernel is ACT-bound (softmax exp = ~166k col-cycles at
1.2 GHz), so everything else is arranged to keep ACT saturated:
  - v^T and the k01 projection+norm run first; attention starts after the
    q01 chunk-0 norm (~10us in).
  - All remaining PE work (q chunks 1-4, pair-1 QKV, norms, output
    projections) is issued as "fillers" interleaved between the j-tiles of
    the attention stream, so the PE never detours long enough to starve ACT
    and the sim pipeline (PSUM bufs=2) stays full.
  - Partition broadcasts ([1,N] -> [64,N]) are K=1 PE matmuls into a PSUM
    tile (onesrow^T x row) instead of DRAM round-trips.
  - softmax 1/s uses the DVE approximate-reciprocal custom op (partition
    base 0 only - custom DVE ops misbehave at base 64 on HW).

On-core layout ([partition, free]):
  q,k "channels-major" [d, n] pairs: tile m in {q01,q23,k01,k23} = [128, N]
  v transposed [n, d] per j-tile (from a separate x^T @ w_v^T matmul).
  sim^T chunk [j, i] = k^T q in PSUM (two heads row-packed via tile_position);
  ACT does exp(PSUM)->SBUF in [128, 1024] instructions (no max subtraction
  needed: |sim|<=10 exactly since q,k are unit vectors).
  softmax denominators via ones-column matmuls accumulated alongside E@v.
  Norm-row layout: ssA holds q norm-squares (q01 h0@row0 h1@row32, q23
  h0@64 h1@96), ssB the same for k — every row at a legal engine partition
  base so broadcasts can be PE matmuls. 1/sqrt is exp(-0.5 ln x) on ACT.
  Softmax denominators land at rows {0,32} of s8 (pair 0) / s8b (pair 1).
"""

import math

import numpy as np

B, C, H, W = 4, 256, 48, 48
HEADS, DIM_HEAD, SCALE = 8, 64, 10.0
INNER = HEADS * DIM_HEAD
N = H * W                      # 2304
NJ = N // 128                  # 18 j-tiles
CHUNKS = [(0, 512), (512, 512), (1024, 512), (1536, 512), (2048, 256)]
EPS = 1e-12

WD_NAME = "bf16"               # working dtype: "bf16" | "f32r" | "f32"

_CACHE = {}


def _pin_act_tables():
    """Force every activation onto the natural_log_exp_and_others set so the
    whole kernel needs exactly one ACT table load (Ln+Exp share that set)."""
    import concourse.bacc as bacc_mod
    if getattr(bacc_mod, "_act_tables_pinned", False):
        return
    orig = bacc_mod.get_activation_tables

    def patched(arch):
        t = orig(arch)
        keep = "natural_log_exp_and_others"
        if keep in t:
            return {name: (funcs if name == keep else set())
                    for name, funcs in t.items()}
        return t

    bacc_mod.get_activation_tables = patched
    bacc_mod._act_tables_pinned = True


def _build(wd_name):
    import concourse.tile as tile
    from concourse import bacc, mybir

    _pin_act_tables()

    F32 = mybir.dt.float32
    F32R = mybir.dt.float32r
    WD = mybir.dt.bfloat16 if wd_name == "bf16" else F32

    def mc(ap):
        # matmul operand cast for the fast-fp32 PE path
        return ap.bitcast(F32R) if wd_name == "f32r" else ap

    Ln = mybir.ActivationFunctionType.Ln
    Exp = mybir.ActivationFunctionType.Exp

    nc = bacc.Bacc("TRN2", target_bir_lowering=False, debug=False,
                   enable_asserts=False, num_devices=8)
    x2 = nc.dram_tensor("x2", [2, 2, 128, N // 2], WD,
                        kind="ExternalInput").ap()
    wqk = nc.dram_tensor("wqk", [2, 128, 512], WD, kind="ExternalInput").ap()
    wvT = nc.dram_tensor("wvT", [2, 128, 256], WD, kind="ExternalInput").ap()
    woT = nc.dram_tensor("woT", [2, 128, 256], WD, kind="ExternalInput").ap()
    ones33 = nc.dram_tensor("ones33", [128, 66], WD,
                            kind="ExternalInput").ap()
    onesrow = nc.dram_tensor("onesrow", [128, 64], F32,
                             kind="ExternalInput").ap()
    # output: per head-pair partial projections, summed on host (with bias)
    y = nc.dram_tensor("y", [2, 2, 128, N], F32, kind="ExternalOutput").ap()

    with tile.TileContext(nc) as tc:
        with tc.tile_pool(name="persist", bufs=1) as P, \
             tc.tile_pool(name="sq", bufs=3) as SQ, \
             tc.tile_pool(name="esb", bufs=12) as ESB, \
             tc.tile_pool(name="yst", bufs=3) as YST, \
             tc.tile_pool(name="psf", bufs=1, space="PSUM") as PSF, \
             tc.tile_pool(name="rsbp", bufs=1, space="PSUM") as RSBP:

            # ---- persistent tiles ----
            x_sb = [P.tile([128, N], WD, tag=f"x{c}", name=f"x{c}")
                    for c in range(2)]
            wqk_sb = [P.tile([128, 512], WD, tag=f"wqk{c}", name=f"wqk{c}")
                      for c in range(2)]
            wvT_sb = [P.tile([128, 256], WD, tag=f"wvT{c}", name=f"wvT{c}")
                      for c in range(2)]
            woT_sb = [P.tile([128, 256], WD, tag=f"woT{c}", name=f"woT{c}")
                      for c in range(2)]
            ones33_sb = P.tile([128, 66], WD, tag="ones33", name="ones33")
            onesrow_sb = P.tile([128, 64], F32, tag="onesrow",
                                name="onesrow")

            # weights first (small), then x in column-halves so the first
            # v^T / k01 matmuls can start after ~2us of DMA
            for c in range(2):
                nc.sync.dma_start(wvT_sb[c][:, :], wvT[c])
                nc.sync.dma_start(wqk_sb[c][:, :], wqk[c])
                nc.sync.dma_start(woT_sb[c][:, :], woT[c])
            nc.sync.dma_start(ones33_sb[:, :], ones33)
            nc.sync.dma_start(onesrow_sb[:, :], onesrow)
            for half in range(2):
                hs = slice(half * (N // 2), (half + 1) * (N // 2))
                for c in range(2):
                    nc.sync.dma_start(x_sb[c][:, hs], x2[c][half])

            biasq = P.tile([128, 1], F32, tag="biasq", name="biasq")
            nc.vector.memset(biasq[:, :], math.log(SCALE))

            qk_sb = [P.tile([128, N], WD, tag=f"qk{m}", name=f"qk{m}")
                     for m in range(4)]
            # norm-square rows: ssA = q (h0@+0, h1@+32), ssB = k; base 0 for
            # pair 0, base 64 for pair 1
            ssA = P.tile([128, N], F32, tag="ssA", name="ssA")
            ssB = P.tile([128, N], F32, tag="ssB", name="ssB")
            nc.vector.memset(ssA[:, :], 1.0)
            nc.vector.memset(ssB[:, :], 1.0)
            ln8 = P.tile([128, N], F32, tag="ln8", name="ln8")
            rsA = P.tile([128, N], F32, tag="rsA", name="rsA")
            rsB = P.tile([128, N], F32, tag="rsB", name="rsB")
            qhat = [P.tile([128, N], WD, tag=f"qh{p}", name=f"qh{p}")
                    for p in range(2)]
            khat = [P.tile([128, N], WD, tag=f"kh{p}", name=f"kh{p}")
                    for p in range(2)]
            vT_sb = P.tile([128, NJ, 4, 64], WD, tag="vT", name="vT")

            numer = [P.tile([128, N], WD, tag=f"nu{p}", name=f"nu{p}")
                     for p in range(2)]
            nsc = [P.tile([128, N], WD, tag=f"nsc{p}", name=f"nsc{p}")
                   for p in range(2)]
            # softmax denominator rows at {0,32}: pair 0 in s8 (aliases ssB),
            # pair 1 in s8b (aliases rsA); 1/s in rsden (aliases rsB). All
            # alias pairs are lifetime-disjoint per (row, column) region.
            s8 = P.tile([128, N], F32, tag="ssB", name="s8")
            s8b = P.tile([128, N], F32, tag="rsA", name="s8b")
            rsden = P.tile([128, N], F32, tag="rsB", name="rsden")

            # m tile -> (ss tile, row base): q01,k01 pair0; q23,k23 pair1
            M_SS = {0: ("A", 0), 2: ("B", 0), 1: ("A", 64), 3: ("B", 64)}

            def sstile(which):
                return ssA if which == "A" else ssB

            def rstile(which):
                return rsA if which == "A" else rsB

            def qkv_mm(m, off, cw, PQ):
                pq = PQ.tile([128, 512], F32, tag="pf", name="pq")
                for c in range(2):
                    nc.tensor.matmul(
                        pq[:, 0:cw],
                        mc(wqk_sb[c][:, m * 128:(m + 1) * 128]),
                        mc(x_sb[c][:, off:off + cw]),
                        start=(c == 0), stop=(c == 1))
                nc.vector.tensor_copy(qk_sb[m][:, off:off + cw],
                                      pq[:, 0:cw])
                q2 = SQ.tile([128, 512], WD, tag="q2", name="q2")
                nc.vector.tensor_mul(q2[:, 0:cw],
                                     qk_sb[m][:, off:off + cw],
                                     qk_sb[m][:, off:off + cw])
                return q2

            def qkv_ss(m, off, cw, q2, PQ):
                which, b0 = M_SS[m]
                pss = PQ.tile([33, 512], F32, tag="pf", name="pss")
                nc.tensor.matmul(pss[:, 0:cw], mc(ones33_sb[:, 0:33]),
                                 mc(q2[:, 0:cw]), start=True, stop=True)
                nc.vector.tensor_copy(sstile(which)[b0:b0 + 33, off:off + cw],
                                      pss[0:33, 0:cw])

            def rs_rows(which, b0, off, cw):
                # rs = SCALE_or_1 / sqrt(max(ss, eps^2)) on 64 partitions
                # (rows b0, b0+32 meaningful): 2 ACT instrs
                ss = sstile(which)
                sl = slice(b0, b0 + 64)
                nc.vector.tensor_scalar_max(ss[sl, off:off + cw],
                                            ss[sl, off:off + cw],
                                            EPS * EPS)
                nc.scalar.activation(ln8[sl, off:off + cw],
                                     ss[sl, off:off + cw], Ln)
                kw = {"bias": biasq[sl, :]} if which == "A" else {}
                nc.scalar.activation(rstile(which)[sl, off:off + cw],
                                     ln8[sl, off:off + cw], Exp,
                                     scale=-0.5, **kw)

            def bcast_mm(rsb, src, b0, off, cw):
                # [1,cw] row -> [64,cw] partition broadcast via K=1 matmul,
                # both head rows into one [128,cw] PSUM tile
                nc.tensor.matmul(rsb[0:64, 0:cw],
                                 onesrow_sb[b0:b0 + 1, 0:64],
                                 src[b0:b0 + 1, off:off + cw],
                                 start=True, stop=True,
                                 tile_position=(b0, 0))
                nc.tensor.matmul(rsb[64:128, 0:cw],
                                 onesrow_sb[b0 + 32:b0 + 33, 0:64],
                                 src[b0 + 32:b0 + 33, off:off + cw],
                                 start=True, stop=True,
                                 tile_position=(b0 + 32, 64))

            def norm_chunk(dst, m, off, cw):
                which, b0 = M_SS[m]
                rsb = RSBP.tile([128, 512], F32, tag="rsbp", name="rsb")
                bcast_mm(rsb, rstile(which), b0, off, cw)
                nc.vector.tensor_mul(dst[:, off:off + cw],
                                     qk_sb[m][:, off:off + cw],
                                     rsb[:, 0:cw])

            def attention_chunk(hp, off, cw, PSSIM, PSO, fillers=()):
                fillers = list(fillers)
                po = PSO.tile([128, 512], F32, tag="po", name="po")
                po_o = PSO.tile([64, 512], F32, tag="po_o", name="po_o")

                def sim_pair(jt, ps):
                    js = slice(jt * 128, (jt + 1) * 128)
                    nc.tensor.matmul(
                        ps[:, 0:cw],
                        mc(khat[hp][0:64, js]),
                        mc(qhat[hp][0:64, off:off + cw]),
                        start=True, stop=True, tile_position=(0, 0))
                    nc.tensor.matmul(
                        ps[:, 512:512 + cw],
                        mc(khat[hp][64:128, js]),
                        mc(qhat[hp][64:128, off:off + cw]),
                        start=True, stop=True, tile_position=(64, 0))

                def ev_group(jt, eh0, eh1):
                    st, sp = (jt == 0), (jt == NJ - 1)
                    nc.tensor.matmul(
                        po[0:64, 0:cw],
                        mc(vT_sb[:, jt, 2 * hp, :]),
                        mc(eh0),
                        start=st, stop=sp, tile_position=(0, 0),
                        skip_group_check=True)
                    nc.tensor.matmul(
                        po[64:128, 0:cw],
                        mc(vT_sb[:, jt, 2 * hp + 1, :]),
                        mc(eh1),
                        start=st, stop=sp, tile_position=(0, 64),
                        skip_group_check=True)
                    nc.tensor.matmul(
                        po_o[0:32, 0:cw],
                        mc(ones33_sb[:, 33:65]),
                        mc(eh0),
                        start=st, stop=sp, tile_position=(0, 0),
                        skip_group_check=True)
                    nc.tensor.matmul(
                        po_o[32:64, 0:cw],
                        mc(ones33_sb[:, 33:65]),
                        mc(eh1),
                        start=st, stop=sp, tile_position=(0, 32),
                        skip_group_check=True)

                # E@v trails 3 j's behind so the next chunk's first E@v
                # (which waits the previous chunk's po drain) never blocks
                # early sims on the in-order PE
                pend = []
                for jt in range(NJ):
                    ps = PSSIM.tile([128, 1024], F32, tag="ps", name="ps")
                    sim_pair(jt, ps)
                    e = ESB.tile([128, 1024], WD, tag="e", name="e")
                    ps3 = ps.rearrange("p (b c) -> p b c", b=2)
                    e3b = e.rearrange("p (b c) -> p b c", b=2)
                    nc.scalar.activation(e3b[:, :, 0:cw],
                                         ps3[:, :, 0:cw], Exp)
                    pend.append((jt, e))
                    if len(pend) > 3:
                        j0, ee = pend.pop(0)
                        ev_group(j0, ee[:, 0:cw], ee[:, 512:512 + cw])
                    if fillers:
                        fillers.pop(0)()
                for (j0, ee) in pend:
                    ev_group(j0, ee[:, 0:cw], ee[:, 512:512 + cw])
                for f in fillers:
                    f()
                # drain numerator + both denominator rows (one 33-row copy)
                nc.vector.tensor_copy(numer[hp][:, off:off + cw],
                                      po[:, 0:cw])
                dsts = s8 if hp == 0 else s8b
                nc.vector.tensor_copy(dsts[0:64, off:off + cw],
                                      po_o[0:64, 0:cw])

            def scale_chunk(hp, off, cw):
                # 1/s on DVE (approx custom op; partition base 0 only),
                # broadcast via matmul, rescale numerators
                src = s8 if hp == 0 else s8b
                nc.vector.reciprocal_approx_fast(
                    out=rsden[0:64, off:off + cw],
                    in_=src[0:64, off:off + cw])
                rsb = RSBP.tile([128, 512], F32, tag="rsbp", name="rsbs")
                bcast_mm(rsb, rsden, 0, off, cw)
                nc.vector.tensor_mul(nsc[hp][:, off:off + cw],
                                     numer[hp][:, off:off + cw],
                                     rsb[:, 0:cw])

            def outproj_m2(pr, m2, off, cw):
                pf = PSF.tile([128, 512], F32, tag="pf", name="pf")
                nc.tensor.matmul(
                    pf[:, 0:cw],
                    mc(woT_sb[pr][:, m2 * 128:(m2 + 1) * 128]),
                    mc(nsc[pr][:, off:off + cw]),
                    start=True, stop=True)
                yt = YST.tile([128, 512], F32, tag="yt", name="yt")
                nc.vector.tensor_copy(yt[:, 0:cw], pf[:, 0:cw])
                nc.sync.dma_start(y[pr][m2][:, off:off + cw],
                                  yt[:, 0:cw])

            # ---- head: v^T, k01 projection + norm, q01 chunk 0 ----
            with tc.tile_pool(name="psq", bufs=2, space="PSUM") as PSQ, \
                 tc.tile_pool(name="psv", bufs=2, space="PSUM") as PSV:
                for jt in range(NJ):
                    pv = PSV.tile([128, 256], F32, tag="pv", name="pv")
                    for c in range(2):
                        nc.tensor.matmul(
                            pv[:, :],
                            mc(x_sb[c][:, jt * 128:(jt + 1) * 128]),
                            mc(wvT_sb[c][:, :]),
                            start=(c == 0), stop=(c == 1))
                    nc.vector.tensor_copy(
                        vT_sb[:, jt, :, :],
                        pv.rearrange("p (h d) -> p h d", h=4))
                for (off, cw) in CHUNKS:
                    q2 = qkv_mm(2, off, cw, PSQ)
                    qkv_ss(2, off, cw, q2, PSQ)
                rs_rows("B", 0, 0, N)
                for (off, cw) in CHUNKS:
                    norm_chunk(khat[0], 2, off, cw)
                off0, cw0 = CHUNKS[0]
                q2 = qkv_mm(0, off0, cw0, PSQ)
                qkv_ss(0, off0, cw0, q2, PSQ)
                rs_rows("A", 0, off0, cw0)
                norm_chunk(qhat[0], 0, off0, cw0)

            with tc.tile_pool(name="pssim", bufs=2, space="PSUM") as PSSIM, \
                 tc.tile_pool(name="pso", bufs=1, space="PSUM") as PSO:

                def qkv_fillers(m, off, cw):
                    st = {}

                    def f1():
                        st["q2"] = qkv_mm(m, off, cw, PSF)

                    def f2():
                        qkv_ss(m, off, cw, st["q2"], PSF)
                    return [f1, f2]

                # ---- hp0 attention; fillers: q01 chunk ci+1 prep and
                # pair-1 QKV (+ pair-1 norms on the last chunk) ----
                for ci, (off, cw) in enumerate(CHUNKS):
                    fillers = []
                    if ci + 1 < len(CHUNKS):
                        off2, cw2 = CHUNKS[ci + 1]
                        fillers += qkv_fillers(0, off2, cw2)
                        fillers.append(
                            lambda o=off2, c=cw2: rs_rows("A", 0, o, c))
                        fillers.append(
                            lambda o=off2, c=cw2:
                            norm_chunk(qhat[0], 0, o, c))
                    fillers += qkv_fillers(1, off, cw)
                    fillers += qkv_fillers(3, off, cw)
                    if ci == len(CHUNKS) - 1:
                        fillers.append(lambda: rs_rows("A", 64, 0, N))
                        fillers.append(lambda: rs_rows("B", 64, 0, N))
                        for (o3, c3) in CHUNKS:
                            fillers.append(
                                lambda o=o3, c=c3:
                                norm_chunk(qhat[1], 1, o, c))
                            fillers.append(
                                lambda o=o3, c=c3:
                                norm_chunk(khat[1], 3, o, c))
                    attention_chunk(0, off, cw, PSSIM, PSO, fillers)

                # ---- hp1 attention; fillers: pair-0 scale+outproj for
                # this chunk (denominators final since hp0), pair-1
                # trailing by one chunk ----
                for ci, (off, cw) in enumerate(CHUNKS):
                    fillers = []
                    if ci > 0:
                        offp, cwp = CHUNKS[ci - 1]
                        fillers.append(
                            lambda o=offp, c=cwp: scale_chunk(1, o, c))
                        fillers.append(
                            lambda o=offp, c=cwp: outproj_m2(1, 0, o, c))
                        fillers.append(
                            lambda o=offp, c=cwp: outproj_m2(1, 1, o, c))
                    fillers.append(lambda o=off, c=cw: scale_chunk(0, o, c))
                    fillers.append(lambda o=off, c=cw: outproj_m2(0, 0, o, c))
                    fillers.append(lambda o=off, c=cw: outproj_m2(0, 1, o, c))
                    attention_chunk(1, off, cw, PSSIM, PSO, fillers)
                # tail: last pair-1 chunk
                offp, cwp = CHUNKS[-1]
                scale_chunk(1, offp, cwp)
                outproj_m2(1, 0, offp, cwp)
                outproj_m2(1, 1, offp, cwp)

    nc.compile()
    return nc


def _get_program(wd_name=WD_NAME):
    if wd_name not in _CACHE:
        _CACHE[wd_name] = _build(wd_name)
    return _CACHE[wd_name]


def _np_wd(wd_name):
    if wd_name == "bf16":
        import ml_dtypes
        return np.dtype(ml_dtypes.bfloat16)
    return np.dtype(np.float32)


def make_in_maps(x, w_qkv, w_out, b_out, wd_name=WD_NAME):
    x = np.asarray(x, np.float32)
    w_qkv = np.asarray(w_qkv, np.float32)
    w_out = np.asarray(w_out, np.float32)
    wd = _np_wd(wd_name)

    ones33 = np.zeros((128, 66), np.float32)
    ones33[0:64, 0] = 1.0     # h-even sum -> pss row 0
    ones33[64:128, 32] = 1.0  # h-odd sum -> pss row 32
    ones33[:, 33] = 1.0       # denominator block: full sum -> row +0
    onesrow = np.ones((128, 64), np.float32)

    in_maps = []
    for core in range(8):
        b, half = core // 2, core % 2
        hsel = slice(256 * half, 256 * (half + 1))
        q_rows = np.arange(0, 512)[hsel]
        k_rows = 512 + q_rows
        v_rows = 1024 + q_rows
        wqk_h = np.ascontiguousarray(
            w_qkv[np.r_[q_rows, k_rows], :].T).reshape(2, 128, 512)
        wvT_h = np.ascontiguousarray(w_qkv[v_rows, :].T).reshape(2, 128, 256)
        woT_h = np.ascontiguousarray(w_out[:, hsel].T).reshape(2, 128, 256)
        in_maps.append({
            "x2": x[b].reshape(C, N).reshape(2, 128, 2, N // 2)
                     .transpose(0, 2, 1, 3).astype(wd),
            "wqk": wqk_h.astype(wd),
            "wvT": wvT_h.astype(wd),
            "woT": woT_h.astype(wd),
            "ones33": ones33.astype(wd),
            "onesrow": onesrow.astype(np.float32),
        })
    return in_maps


def gather_output(results, b_out):
    outs = [r["y"].sum(axis=0).reshape(C, N) for r in results]
    bias = np.asarray(b_out, np.float32).reshape(C, 1)
    return np.stack([
        (outs[2 * b] + outs[2 * b + 1] + bias).reshape(C, H, W)
        for b in range(B)
    ]).astype(np.float32)


def run(in_maps, wd_name=WD_NAME, **kwargs):
    from concourse import bass_utils
    nc = _get_program(wd_name)
    return bass_utils.run_bass_kernel_spmd(nc, in_maps,
                                           core_ids=list(range(8)), **kwargs)


def kernel(x, w_qkv, w_out, b_out):
    in_maps = make_in_maps(x, w_qkv, w_out, b_out)
    res = run(in_maps)
    return gather_output(res.results, b_out)


# revision 21
# speedup vs baseline: 1.0308x; 1.0308x over previous
"""Trainium2 Bass kernel for nn_Attention_21895743275585.

Reference computation (per batch b of 4):
  qkv = w_qkv @ x_flat            # 1x1 conv, x_flat [C=256, N=2304]
  q,k l2-normalized per (head, n) along dim_head=64; SCALE=10
  sim = 10 * qhat^T khat per head; attn = softmax(sim, axis=-1)
  out = attn @ v; final = w_out @ out_inner + b_out

Sharding: 8 cores = (batch b, head-half). Each core handles 4 of the 8 heads
of one batch; host sums the partial output projections and adds b_out.

Schedule (v3): the kernel is ACT-bound (softmax exp = ~166k col-cycles at
1.2 GHz), so everything else is arranged to keep ACT saturated:
  - v^T and the k01 projection+norm run first; attention starts after the
    q01 chunk-0 norm (~10us in).
  - All remaining PE work (q chunks 1-4, pair-1 QKV, norms, output
    projections) is issued as "fillers" interleaved between the j-tiles of
    the attention stream, so the PE never detours long enough to starve ACT
    and the sim pipeline (PSUM bufs=2) stays full.
  - Partition broadcasts ([1,N] -> [64,N]) are K=1 PE matmuls into a PSUM
    tile (onesrow^T x row) instead of DRAM round-trips.
  - softmax 1/s uses the DVE approximate-reciprocal custom op (partition
    base 0 only - custom DVE ops misbehave at base 64 on HW).

On-core layout ([partition, free]):
  q,k "channels-major" [d, n] pairs: tile m in {q01,q23,k01,k23} = [128, N]
  v transposed [n, d] per j-tile (from a separate x^T @ w_v^T matmul).
  sim^T chunk [j, i] = k^T q in PSUM (two heads row-packed via tile_position);
  ACT does exp(PSUM)->SBUF in [128, 1024] instructions (no max subtraction
  needed: |sim|<=10 exactly since q,k are unit vectors).
  softmax denominators via ones-column matmuls accumulated alongside E@v.
  Norm-row layout: ssA holds q norm-squares (q01 h0@row0 h1@row32, q23
  h0@64 h1@96), ssB the same for k — every row at a legal engine partition
  base so broadcasts can be PE matmuls. 1/sqrt is exp(-0.5 ln x) on ACT.
  Softmax denominators land at rows {0,32} of s8 (pair 0) / s8b (pair 1).
"""

import math

import numpy as np

B, C, H, W = 4, 256, 48, 48
HEADS, DIM_HEAD, SCALE = 8, 64, 10.0
INNER = HEADS * DIM_HEAD
N = H * W                      # 2304
NJ = N // 128                  # 18 j-tiles
CHUNKS = [(0, 512), (512, 512), (1024, 512), (1536, 512), (2048, 256)]
EPS = 1e-12

WD_NAME = "bf16"               # working dtype: "bf16" | "f32r" | "f32"

_CACHE = {}


def _pin_act_tables():
    """Force every activation onto the natural_log_exp_and_others set so the
    whole kernel needs exactly one ACT table load (Ln+Exp share that set)."""
    import concourse.bacc as bacc_mod
    if getattr(bacc_mod, "_act_tables_pinned", False):
        return
    orig = bacc_mod.get_activation_tables

    def patched(arch):
        t = orig(arch)
        keep = "natural_log_exp_and_others"
        if keep in t:
            return {name: (funcs if name == keep else set())
                    for name, funcs in t.items()}
        return t

    bacc_mod.get_activation_tables = patched
    bacc_mod._act_tables_pinned = True


def _build(wd_name):
    import concourse.bass as bass
    import concourse.tile as tile
    from concourse import bacc, mybir

    _pin_act_tables()

    F32 = mybir.dt.float32
    F32R = mybir.dt.float32r
    WD = mybir.dt.bfloat16 if wd_name == "bf16" else F32

    def mc(ap):
        # matmul operand cast for the fast-fp32 PE path
        return ap.bitcast(F32R) if wd_name == "f32r" else ap

    Ln = mybir.ActivationFunctionType.Ln
    Exp = mybir.ActivationFunctionType.Exp

    nc = bacc.Bacc("TRN2", target_bir_lowering=False, debug=False,
                   enable_asserts=False, num_devices=8)
    x2 = nc.dram_tensor("x2", [2, 2, 128, N // 2], WD,
                        kind="ExternalInput").ap()
    wqk = nc.dram_tensor("wqk", [2, 128, 512], WD, kind="ExternalInput").ap()
    wvT = nc.dram_tensor("wvT", [2, 128, 256], WD, kind="ExternalInput").ap()
    woT = nc.dram_tensor("woT", [2, 128, 256], WD, kind="ExternalInput").ap()
    ones33 = nc.dram_tensor("ones33", [128, 66], WD,
                            kind="ExternalInput").ap()
    # internal DRAM bounce rows for partition broadcasts (DRAM APs allow a
    # step-0 partition dim): 8 norm rows + 8 denominator rows
    rsd = nc.dram_tensor("rsd", [16, N], F32, kind="Internal").ap()
    # output: per head-pair partial projections, summed on host (with bias)
    y = nc.dram_tensor("y", [2, 2, 128, N], F32, kind="ExternalOutput").ap()

    with tile.TileContext(nc) as tc:
        with tc.tile_pool(name="persist", bufs=1) as P, \
             tc.tile_pool(name="sq", bufs=3) as SQ, \
             tc.tile_pool(name="esb", bufs=12) as ESB, \
             tc.tile_pool(name="yst", bufs=3) as YST, \
             tc.tile_pool(name="rsb", bufs=3) as RSB, \
             tc.tile_pool(name="psf", bufs=2, space="PSUM") as PSF:

            # ---- persistent tiles ----
            x_sb = [P.tile([128, N], WD, tag=f"x{c}", name=f"x{c}")
                    for c in range(2)]
            wqk_sb = [P.tile([128, 512], WD, tag=f"wqk{c}", name=f"wqk{c}")
                      for c in range(2)]
            wvT_sb = [P.tile([128, 256], WD, tag=f"wvT{c}", name=f"wvT{c}")
                      for c in range(2)]
            woT_sb = [P.tile([128, 256], WD, tag=f"woT{c}", name=f"woT{c}")
                      for c in range(2)]
            ones33_sb = P.tile([128, 66], WD, tag="ones33", name="ones33")


            # k01 inputs first (wqk + x) so its matmuls start earliest,
            # then v^T weights, then the rest
            for c in range(2):
                nc.sync.dma_start(wqk_sb[c][:, :], wqk[c])
            nc.sync.dma_start(ones33_sb[:, :], ones33)
            for half in range(2):
                hs = slice(half * (N // 2), (half + 1) * (N // 2))
                for c in range(2):
                    nc.sync.dma_start(x_sb[c][:, hs], x2[c][half])
            for c in range(2):
                nc.sync.dma_start(wvT_sb[c][:, :], wvT[c])
            for c in range(2):
                nc.sync.dma_start(woT_sb[c][:, :], woT[c])

            biasq = P.tile([128, 1], F32, tag="biasq", name="biasq")
            nc.vector.memset(biasq[:, :], math.log(SCALE))

            qk_sb = [P.tile([128, N], WD, tag=f"qk{m}", name=f"qk{m}")
                     for m in range(4)]
            # norm-square rows: ssA = q (h0@+0, h1@+32), ssB = k; base 0 for
            # pair 0, base 64 for pair 1
            ssA = P.tile([128, N], F32, tag="ssA", name="ssA")
            ssB = P.tile([128, N], F32, tag="ssB", name="ssB")
            nc.vector.memset(ssA[:, :], 1.0)
            nc.vector.memset(ssB[:, :], 1.0)
            ln8 = P.tile([128, N], F32, tag="ln8", name="ln8")
            rsA = P.tile([128, N], F32, tag="rsA", name="rsA")
            rsB = P.tile([128, N], F32, tag="rsB", name="rsB")
            qhat = [P.tile([128, N], WD, tag=f"qh{p}", name=f"qh{p}")
                    for p in range(2)]
            khat = [P.tile([128, N], WD, tag=f"kh{p}", name=f"kh{p}")
                    for p in range(2)]
            vT_sb = P.tile([128, NJ, 4, 64], WD, tag="vT", name="vT")

            numer = [P.tile([128, N], WD, tag=f"nu{p}", name=f"nu{p}")
                     for p in range(2)]
            nsc = [P.tile([128, N], WD, tag=f"nsc{p}", name=f"nsc{p}")
                   for p in range(2)]
            # softmax denominator rows at {0,32}: pair 0 in s8, pair 1 in
            # s8b; 1/s in rsden (both pairs, column-phased)
            s8 = P.tile([128, N], F32, tag="s8", name="s8")
            s8b = P.tile([128, N], F32, tag="s8b", name="s8b")
            rsden = P.tile([128, N], F32, tag="rsden", name="rsden")

            # m tile -> (ss tile, row base): q01,k01 pair0; q23,k23 pair1
            M_SS = {0: ("A", 0), 2: ("B", 0), 1: ("A", 64), 3: ("B", 64)}

            def sstile(which):
                return ssA if which == "A" else ssB

            def rstile(which):
                return rsA if which == "A" else rsB

            def qkv_mm(m, off, cw, PQ):
                pq = PQ.tile([128, 512], F32, tag="pf", name="pq")
                for c in range(2):
                    nc.tensor.matmul(
                        pq[:, 0:cw],
                        mc(wqk_sb[c][:, m * 128:(m + 1) * 128]),
                        mc(x_sb[c][:, off:off + cw]),
                        start=(c == 0), stop=(c == 1))
                nc.vector.tensor_copy(qk_sb[m][:, off:off + cw],
                                      pq[:, 0:cw])
                q2 = SQ.tile([128, 512], WD, tag="q2", name="q2")
                nc.vector.tensor_mul(q2[:, 0:cw],
                                     qk_sb[m][:, off:off + cw],
                                     qk_sb[m][:, off:off + cw])
                return q2

            def qkv_ss(m, off, cw, q2, PQ):
                which, b0 = M_SS[m]
                pss = PQ.tile([33, 512], F32, tag="pf", name="pss")
                nc.tensor.matmul(pss[:, 0:cw], mc(ones33_sb[:, 0:33]),
                                 mc(q2[:, 0:cw]), start=True, stop=True)
                nc.vector.tensor_copy(sstile(which)[b0:b0 + 33, off:off + cw],
                                      pss[0:33, 0:cw])

            def rs_rows(which, b0, off, cw):
                # rs = SCALE_or_1 / sqrt(max(ss, eps^2)) on 64 partitions
                # (rows b0, b0+32 meaningful): 2 ACT instrs
                ss = sstile(which)
                sl = slice(b0, b0 + 64)
                nc.vector.tensor_scalar_max(ss[sl, off:off + cw],
                                            ss[sl, off:off + cw],
                                            EPS * EPS)
                nc.scalar.activation(ln8[sl, off:off + cw],
                                     ss[sl, off:off + cw], Ln)
                kw = {"bias": biasq[sl, :]} if which == "A" else {}
                nc.scalar.activation(rstile(which)[sl, off:off + cw],
                                     ln8[sl, off:off + cw], Exp,
                                     scale=-0.5, **kw)

            def bcast_row(dram_row_ap, dst_ap, parts):
                src = bass.AP(tensor=dram_row_ap.tensor,
                              offset=dram_row_ap.offset,
                              ap=[[0, parts]] + list(dram_row_ap.ap))
                nc.sync.dma_start(dst_ap, src)

            def bounce(src, b0, drow, off, cw, rsb):
                # [1,cw] rows b0 / b0+32 -> [64,cw] halves of rsb via a DRAM
                # round trip (engine-free; latency hidden by the stream)
                nc.sync.dma_start(rsd[drow:drow + 1, off:off + cw],
                                  src[b0:b0 + 1, off:off + cw])
                nc.sync.dma_start(rsd[drow + 1:drow + 2, off:off + cw],
                                  src[b0 + 32:b0 + 33, off:off + cw])
                bcast_row(rsd[drow][off:off + cw],
                          rsb[0:64, 0:cw], 64)
                bcast_row(rsd[drow + 1][off:off + cw],
                          rsb[64:128, 0:cw], 64)

            def norm_chunk(dst, m, off, cw):
                which, b0 = M_SS[m]
                drow = {0: 0, 2: 2, 1: 4, 3: 6}[m]
                rsb = RSB.tile([128, 512], F32, tag="rsb", name="rsb")
                bounce(rstile(which), b0, drow, off, cw, rsb)
                nc.vector.tensor_mul(dst[:, off:off + cw],
                                     qk_sb[m][:, off:off + cw],
                                     rsb[:, 0:cw])

            def attention_stream(seq, fillers_by_key, PSSIM, PSO):
                """One software-pipelined stream over all (hp, chunk, jt):
                sim+exp at each step, E@v trailing 3 steps (crossing chunk
                boundaries so ACT never starves), per-chunk drains issued
                right after the chunk's last E@v; one filler per step."""
                state = {}
                pend = []
                fq = []

                def sim_pair(hp, off, cw, jt, ps):
                    js = slice(jt * 128, (jt + 1) * 128)
                    nc.tensor.matmul(
                        ps[:, 0:cw],
                        mc(khat[hp][0:64, js]),
                        mc(qhat[hp][0:64, off:off + cw]),
                        start=True, stop=True, tile_position=(0, 0))
                    nc.tensor.matmul(
                        ps[:, 512:512 + cw],
                        mc(khat[hp][64:128, js]),
                        mc(qhat[hp][64:128, off:off + cw]),
                        start=True, stop=True, tile_position=(64, 0))

                def ev_issue(hp, ci, off, cw, jt, e):
                    eh0, eh1 = e[:, 0:cw], e[:, 512:512 + cw]
                    stt = state[(hp, ci)]
                    po, po_o = stt["po"], stt["po_o"]
                    st, sp = (jt == 0), (jt == NJ - 1)
                    nc.tensor.matmul(
                        po[0:64, 0:cw],
                        mc(vT_sb[:, jt, 2 * hp, :]),
                        mc(eh0),
                        start=st, stop=sp, tile_position=(0, 0),
                        skip_group_check=True)
                    nc.tensor.matmul(
                        po[64:128, 0:cw],
                        mc(vT_sb[:, jt, 2 * hp + 1, :]),
                        mc(eh1),
                        start=st, stop=sp, tile_position=(0, 64),
                        skip_group_check=True)
                    nc.tensor.matmul(
                        po_o[0:32, 0:cw],
                        mc(ones33_sb[:, 33:65]),
                        mc(eh0),
                        start=st, stop=sp, tile_position=(0, 0),
                        skip_group_check=True)
                    nc.tensor.matmul(
                        po_o[32:64, 0:cw],
                        mc(ones33_sb[:, 33:65]),
                        mc(eh1),
                        start=st, stop=sp, tile_position=(0, 32),
                        skip_group_check=True)
                    if jt == NJ - 1:
                        nc.vector.tensor_copy(numer[hp][:, off:off + cw],
                                              po[:, 0:cw])
                        dsts = s8 if hp == 0 else s8b
                        nc.vector.tensor_copy(dsts[0:64, off:off + cw],
                                              po_o[0:64, 0:cw])

                for (hp, ci, off, cw) in seq:
                    for jt in range(NJ):
                        if jt == 0:
                            state[(hp, ci)] = {
                                "po": PSO.tile([128, 512], F32, tag="po",
                                               name="po"),
                                "po_o": PSO.tile([64, 512], F32,
                                                 tag="po_o", name="po_o"),
                            }
                            fq.extend(fillers_by_key.get((hp, ci), []))
                        ps = PSSIM.tile([128, 1024], F32, tag="ps",
                                        name="ps")
                        sim_pair(hp, off, cw, jt, ps)
                        e = ESB.tile([128, 1024], WD, tag="e", name="e")
                        ps3 = ps.rearrange("p (b c) -> p b c", b=2)
                        e3b = e.rearrange("p (b c) -> p b c", b=2)
                        nc.scalar.activation(e3b[:, :, 0:cw],
                                             ps3[:, :, 0:cw], Exp)
                        pend.append((hp, ci, off, cw, jt, e))
                        if len(pend) > 3:
                            ev_issue(*pend.pop(0))
                        if fq:
                            fq.pop(0)()
                for item in pend:
                    ev_issue(*item)
                pend.clear()
                for f in fq:
                    f()
                fq.clear()

            def scale_chunk(hp, off, cw):
                # 1/s on DVE (approx custom op; partition base 0 only),
                # broadcast via matmul, rescale numerators
                src = s8 if hp == 0 else s8b
                nc.vector.reciprocal_approx_fast(
                    out=rsden[0:64, off:off + cw],
                    in_=src[0:64, off:off + cw])
                rsb = RSB.tile([128, 512], F32, tag="rsb", name="rsbs")
                bounce(rsden, 0, 8 + 2 * hp, off, cw, rsb)
                nc.vector.tensor_mul(nsc[hp][:, off:off + cw],
                                     numer[hp][:, off:off + cw],
                                     rsb[:, 0:cw])

            def outproj_m2(pr, m2, off, cw):
                pf = PSF.tile([128, 512], F32, tag="pf", name="pf")
                nc.tensor.matmul(
                    pf[:, 0:cw],
                    mc(woT_sb[pr][:, m2 * 128:(m2 + 1) * 128]),
                    mc(nsc[pr][:, off:off + cw]),
                    start=True, stop=True)
                yt = YST.tile([128, 512], F32, tag="yt", name="yt")
                nc.vector.tensor_copy(yt[:, 0:cw], pf[:, 0:cw])
                nc.sync.dma_start(y[pr][m2][:, off:off + cw],
                                  yt[:, 0:cw])

            # ---- head: k01 projection first (its norm chain runs on
            # ACT/DVE while v^T fills the PE), then q01 chunk 0 ----
            with tc.tile_pool(name="psq", bufs=2, space="PSUM") as PSQ, \
                 tc.tile_pool(name="psv", bufs=2, space="PSUM") as PSV:
                for (off, cw) in CHUNKS:
                    q2 = qkv_mm(2, off, cw, PSQ)
                    qkv_ss(2, off, cw, q2, PSQ)
                rs_rows("B", 0, 0, N)
                for jt in range(NJ):
                    pv = PSV.tile([128, 256], F32, tag="pv", name="pv")
                    for c in range(2):
                        nc.tensor.matmul(
                            pv[:, :],
                            mc(x_sb[c][:, jt * 128:(jt + 1) * 128]),
                            mc(wvT_sb[c][:, :]),
                            start=(c == 0), stop=(c == 1))
                    nc.vector.tensor_copy(
                        vT_sb[:, jt, :, :],
                        pv.rearrange("p (h d) -> p h d", h=4))
                    if jt == 3:
                        off0, cw0 = CHUNKS[0]
                        q2 = qkv_mm(0, off0, cw0, PSQ)
                        qkv_ss(0, off0, cw0, q2, PSQ)
                    if jt == 6:
                        for (off, cw) in CHUNKS:
                            norm_chunk(khat[0], 2, off, cw)
                    if jt == 8:
                        off0, cw0 = CHUNKS[0]
                        rs_rows("A", 0, off0, cw0)
                    if jt == 10:
                        off0, cw0 = CHUNKS[0]
                        norm_chunk(qhat[0], 0, off0, cw0)

            with tc.tile_pool(name="pssim", bufs=2, space="PSUM") as PSSIM, \
                 tc.tile_pool(name="pso", bufs=1, space="PSUM") as PSO:

                def qkv_fillers(m, off, cw):
                    st = {}

                    def f1():
                        st["q2"] = qkv_mm(m, off, cw, PSF)

                    def f2():
                        qkv_ss(m, off, cw, st["q2"], PSF)
                    return [f1, f2]

                fillers_by_key = {}
                for ci, (off, cw) in enumerate(CHUNKS):
                    fillers = []
                    if ci + 1 < len(CHUNKS):
                        off2, cw2 = CHUNKS[ci + 1]
                        fillers += qkv_fillers(0, off2, cw2)
                        fillers.append(
                            lambda o=off2, c=cw2: rs_rows("A", 0, o, c))
                        fillers.append(
                            lambda o=off2, c=cw2:
                            norm_chunk(qhat[0], 0, o, c))
                    fillers += qkv_fillers(1, off, cw)
                    fillers += qkv_fillers(3, off, cw)
                    if ci == len(CHUNKS) - 1:
                        fillers.append(lambda: rs_rows("A", 64, 0, N))
                        fillers.append(lambda: rs_rows("B", 64, 0, N))
                        for (o3, c3) in CHUNKS:
                            fillers.append(
                                lambda o=o3, c=c3:
                                norm_chunk(qhat[1], 1, o, c))
                            fillers.append(
                                lambda o=o3, c=c3:
                                norm_chunk(khat[1], 3, o, c))
                    fillers_by_key[(0, ci)] = fillers
                for ci, (off, cw) in enumerate(CHUNKS):
                    # pair-0 work first: its denominators are final since
                    # hp0. pair-1's chunk ci-1 drain is only ISSUED 3 steps
                    # into chunk ci (E@v trails), so scale1 must come at
                    # filler slot >= 3.
                    fillers = [
                        lambda o=off, c=cw: scale_chunk(0, o, c),
                        lambda o=off, c=cw: outproj_m2(0, 0, o, c),
                        lambda o=off, c=cw: outproj_m2(0, 1, o, c),
                    ]
                    if ci > 0:
                        offp, cwp = CHUNKS[ci - 1]
                        fillers.append(
                            lambda o=offp, c=cwp: scale_chunk(1, o, c))
                        fillers.append(
                            lambda o=offp, c=cwp: outproj_m2(1, 0, o, c))
                        fillers.append(
                            lambda o=offp, c=cwp: outproj_m2(1, 1, o, c))
                    fillers_by_key[(1, ci)] = fillers
                seq = [(hp, ci, off, cw)
                       for hp in range(2)
                       for ci, (off, cw) in enumerate(CHUNKS)]
                attention_stream(seq, fillers_by_key, PSSIM, PSO)
                # tail: last pair-1 chunk
                offp, cwp = CHUNKS[-1]
                scale_chunk(1, offp, cwp)
                outproj_m2(1, 0, offp, cwp)
                outproj_m2(1, 1, offp, cwp)

    nc.compile()
    return nc


def _get_program(wd_name=WD_NAME):
    if wd_name not in _CACHE:
        _CACHE[wd_name] = _build(wd_name)
    return _CACHE[wd_name]


def _np_wd(wd_name):
    if wd_name == "bf16":
        import ml_dtypes
        return np.dtype(ml_dtypes.bfloat16)
    return np.dtype(np.float32)


def make_in_maps(x, w_qkv, w_out, b_out, wd_name=WD_NAME):
    x = np.asarray(x, np.float32)
    w_qkv = np.asarray(w_qkv, np.float32)
    w_out = np.asarray(w_out, np.float32)
    wd = _np_wd(wd_name)

    ones33 = np.zeros((128, 66), np.float32)
    ones33[0:64, 0] = 1.0     # h-even sum -> pss row 0
    ones33[64:128, 32] = 1.0  # h-odd sum -> pss row 32
    ones33[:, 33] = 1.0       # denominator block: full sum -> row +0

    in_maps = []
    for core in range(8):
        b, half = core // 2, core % 2
        hsel = slice(256 * half, 256 * (half + 1))
        q_rows = np.arange(0, 512)[hsel]
        k_rows = 512 + q_rows
        v_rows = 1024 + q_rows
        wqk_h = np.ascontiguousarray(
            w_qkv[np.r_[q_rows, k_rows], :].T).reshape(2, 128, 512)
        wvT_h = np.ascontiguousarray(w_qkv[v_rows, :].T).reshape(2, 128, 256)
        woT_h = np.ascontiguousarray(w_out[:, hsel].T).reshape(2, 128, 256)
        in_maps.append({
            "x2": x[b].reshape(C, N).reshape(2, 128, 2, N // 2)
                     .transpose(0, 2, 1, 3).astype(wd),
            "wqk": wqk_h.astype(wd),
            "wvT": wvT_h.astype(wd),
            "woT": woT_h.astype(wd),
            "ones33": ones33.astype(wd),
        })
    return in_maps


def gather_output(results, b_out):
    outs = [r["y"].sum(axis=0).reshape(C, N) for r in results]
    bias = np.asarray(b_out, np.float32).reshape(C, 1)
    return np.stack([
        (outs[2 * b] + outs[2 * b + 1] + bias).reshape(C, H, W)
        for b in range(B)
    ]).astype(np.float32)


def run(in_maps, wd_name=WD_NAME, **kwargs):
    from concourse import bass_utils
    nc = _get_program(wd_name)
    return bass_utils.run_bass_kernel_spmd(nc, in_maps,
                                           core_ids=list(range(8)), **kwargs)


def kernel(x, w_qkv, w_out, b_out):
    in_maps = make_in_maps(x, w_qkv, w_out, b_out)
    res = run(in_maps)
    return gather_output(res.results, b_out)


# revision 22
# speedup vs baseline: 1.2372x; 1.2002x over previous
"""Trainium2 Bass kernel for nn_Attention_21895743275585.

Reference computation (per batch b of 4):
  qkv = w_qkv @ x_flat            # 1x1 conv, x_flat [C=256, N=2304]
  q,k l2-normalized per (head, n) along dim_head=64; SCALE=10
  sim = 10 * qhat^T khat per head; attn = softmax(sim, axis=-1)
  out = attn @ v; final = w_out @ out_inner + b_out

Sharding: 8 cores = (batch b, head-half). Each core handles 4 of the 8 heads
of one batch; host sums the partial output projections (2 halves x 2 head
pairs per batch; bias is fed only to half 0 / pair 0).

On-core layout ([partition, free]):
  q,k "channels-major" [d, n] pairs: tile m in {q01,q23,k01,k23} = [128, N]
  v transposed [n, d] per j-tile (from a separate x^T @ w_v^T matmul) with a
  ones column appended so the E@v matmul also yields softmax denominators.
  sim^T chunk [j, i] = k^T q in PSUM (two heads row-packed via tile_position);
  ACT does exp(PSUM)->SBUF in [128, 1024] instructions (no max subtraction
  needed: |sim|<=10 exactly since q,k are unit vectors).
  1/sqrt and 1/x are computed as exp(-0.5 ln x) / exp(-ln x) -- Ln and Exp
  share one ACT table set (pinned to natural_log_exp_and_others).
  Norm rows live at partition bases {0,32,64,96} of [128, N] tiles (engine
  SBUF APs must start at partition 0/32/64/96); [1,N]->[64,N] partition
  broadcasts bounce through small internal DRAM tensors (DRAM APs allow a
  step-0 partition dim).
"""

import math

import numpy as np

B, C, H, W = 4, 256, 48, 48
HEADS, DIM_HEAD, SCALE = 8, 64, 10.0
INNER = HEADS * DIM_HEAD
N = H * W                      # 2304
NJ = N // 128                  # 18 j-tiles
CHUNKS = [(0, 512), (512, 512), (1024, 512), (1536, 512), (2048, 256)]
EPS = 1e-12

WD_NAME = "bf16"               # working dtype: "bf16" | "f32r" | "f32"

_CACHE = {}


def _pin_act_tables():
    """Force every activation onto the natural_log_exp_and_others set so the
    whole kernel needs exactly one ACT table load (Ln+Exp share that set)."""
    import concourse.bacc as bacc_mod
    if getattr(bacc_mod, "_act_tables_pinned", False):
        return
    orig = bacc_mod.get_activation_tables

    def patched(arch):
        t = orig(arch)
        keep = "natural_log_exp_and_others"
        if keep in t:
            return {name: (funcs if name == keep else set())
                    for name, funcs in t.items()}
        return t

    bacc_mod.get_activation_tables = patched
    bacc_mod._act_tables_pinned = True


def _build(wd_name):
    import concourse.bass as bass
    import concourse.tile as tile
    from concourse import bacc, mybir

    _pin_act_tables()

    F32 = mybir.dt.float32
    F32R = mybir.dt.float32r
    WD = mybir.dt.bfloat16 if wd_name == "bf16" else F32

    def mc(ap):
        # matmul operand cast for the fast-fp32 PE path
        return ap.bitcast(F32R) if wd_name == "f32r" else ap

    Ln = mybir.ActivationFunctionType.Ln
    Exp = mybir.ActivationFunctionType.Exp
    ActCopy = mybir.ActivationFunctionType.Copy

    nc = bacc.Bacc("TRN2", target_bir_lowering=False, debug=False,
                   enable_asserts=False, num_devices=8)
    x2 = nc.dram_tensor("x2", [2, 128, N], WD, kind="ExternalInput").ap()
    wqk = nc.dram_tensor("wqk", [2, 128, 512], WD, kind="ExternalInput").ap()
    wvT = nc.dram_tensor("wvT", [2, 128, 256], WD, kind="ExternalInput").ap()
    woT = nc.dram_tensor("woT", [2, 128, 256], WD, kind="ExternalInput").ap()
    bias = nc.dram_tensor("bias", [2, 128, 1], F32, kind="ExternalInput").ap()
    ones8 = nc.dram_tensor("ones8", [128, 9], WD, kind="ExternalInput").ap()
    # output: per head-pair partial projections, summed on host
    y = nc.dram_tensor("y", [2, 2, 128, N], F32, kind="ExternalOutput").ap()
    # internal DRAM bounce rows for partition broadcasts
    rsd = nc.dram_tensor("rsd", [8, N], F32, kind="Internal").ap()
    rsdd = nc.dram_tensor("rsdd", [4, N], F32, kind="Internal").ap()

    def bcast_row(dram_row_ap, dst_ap, parts):
        src = bass.AP(tensor=dram_row_ap.tensor, offset=dram_row_ap.offset,
                      ap=[[0, parts]] + list(dram_row_ap.ap))
        nc.sync.dma_start(dst_ap, src)

    # m tile -> norm-row base index a: q01->0, k01->1, q23->2, k23->3
    M_OF = [(0, 0), (2, 1), (1, 2), (3, 3)]

    with tile.TileContext(nc) as tc:
        with tc.tile_pool(name="persist", bufs=1) as P, \
             tc.tile_pool(name="bcast", bufs=2) as RSB, \
             tc.tile_pool(name="sq", bufs=3) as SQ, \
             tc.tile_pool(name="esb", bufs=12) as ESB, \
             tc.tile_pool(name="yst", bufs=3) as YST, \
             tc.tile_pool(name="psf", bufs=2, space="PSUM") as PSF:

            # ---- load inputs ----
            x_sb = [P.tile([128, N], WD, tag=f"x{c}", name=f"x{c}")
                    for c in range(2)]
            wqk_sb = [P.tile([128, 512], WD, tag=f"wqk{c}", name=f"wqk{c}")
                      for c in range(2)]
            wvT_sb = [P.tile([128, 256], WD, tag=f"wvT{c}", name=f"wvT{c}")
                      for c in range(2)]
            woT_sb = [P.tile([128, 256], WD, tag=f"woT{c}", name=f"woT{c}")
                      for c in range(2)]
            bias_sb = [P.tile([128, 1], F32, tag=f"bias{c}", name=f"bias{c}")
                       for c in range(2)]
            ones8_sb = P.tile([128, 9], WD, tag="ones8", name="ones8")
            for c in range(2):
                nc.sync.dma_start(x_sb[c][:, :], x2[c])
                nc.sync.dma_start(wqk_sb[c][:, :], wqk[c])
                nc.sync.dma_start(wvT_sb[c][:, :], wvT[c])
                nc.sync.dma_start(woT_sb[c][:, :], woT[c])
                nc.sync.dma_start(bias_sb[c][:, :], bias[c])
            nc.sync.dma_start(ones8_sb[:, :], ones8)

            # per-partition Exp bias: ln(SCALE) on q rows (bases 0, 64),
            # 0 on k rows (bases 32, 96)
            biasln = P.tile([128, 1], F32, tag="biasln", name="biasln")
            nc.vector.memset(biasln[0:32, :], math.log(SCALE))
            nc.vector.memset(biasln[32:64, :], 0.0)
            nc.vector.memset(biasln[64:96, :], math.log(SCALE))
            nc.vector.memset(biasln[96:128, :], 0.0)

            qk_sb = [P.tile([128, N], WD, tag=f"qk{m}", name=f"qk{m}")
                     for m in range(4)]
            ss8 = P.tile([128, N], F32, tag="ss8", name="ss8")
            ln8 = P.tile([128, N], F32, tag="ln8", name="ln8")
            rs8 = P.tile([128, N], F32, tag="rs8", name="rs8")
            nc.vector.memset(ss8[:, :], 1.0)
            qhat = [P.tile([128, N], WD, tag=f"qh{p}", name=f"qh{p}")
                    for p in range(2)]
            khat = [P.tile([128, N], WD, tag=f"kh{p}", name=f"kh{p}")
                    for p in range(2)]
            vT_sb = P.tile([128, NJ, 4, 64], WD, tag="vT", name="vT")

            numer = [P.tile([128, N], WD, tag=f"nu{p}", name=f"nu{p}")
                     for p in range(2)]
            nsc = [P.tile([128, N], WD, tag=f"nsc{p}", name=f"nsc{p}")
                   for p in range(2)]
            s8 = P.tile([128, N], F32, tag="ss8", name="s8")
            lnd8 = P.tile([128, N], F32, tag="ln8", name="lnd8")
            rsden8 = P.tile([128, N], F32, tag="rs8", name="rsden8")
            nc.vector.memset(s8[:, :], 1.0)

            # ---- phase 1: QKV projection, norms, v^T ----
            with tc.tile_pool(name="psq", bufs=2, space="PSUM") as PSQ, \
                 tc.tile_pool(name="pss", bufs=2, space="PSUM") as PSS, \
                 tc.tile_pool(name="psv", bufs=2, space="PSUM") as PSV:

                def qkv_chunk(m, a, off, cw, copy_eng, PQ, PS2,
                              pqtag, psstag):
                    base = 32 * a
                    pq = PQ.tile([128, 512], F32, tag=pqtag, name=pqtag)
                    for c in range(2):
                        nc.tensor.matmul(
                            pq[:, 0:cw],
                            mc(wqk_sb[c][:, m * 128:(m + 1) * 128]),
                            mc(x_sb[c][:, off:off + cw]),
                            start=(c == 0), stop=(c == 1))
                    if copy_eng == "act":
                        nc.scalar.activation(qk_sb[m][:, off:off + cw],
                                             pq[:, 0:cw], ActCopy)
                    else:
                        nc.vector.tensor_copy(qk_sb[m][:, off:off + cw],
                                              pq[:, 0:cw])
                    q2 = SQ.tile([128, 512], WD, tag="q2", name="q2")
                    nc.vector.tensor_mul(q2[:, 0:cw],
                                         qk_sb[m][:, off:off + cw],
                                         qk_sb[m][:, off:off + cw])
                    pss = PS2.tile([8, 512], F32, tag=psstag, name=psstag)
                    nc.tensor.matmul(pss[:, 0:cw], mc(ones8_sb[:, 0:8]),
                                     mc(q2[:, 0:cw]), start=True, stop=True)
                    nc.vector.tensor_copy(ss8[base:base + 2, off:off + cw],
                                          pss[0:2, 0:cw])

                def rs_batched(p):
                    # whole-row: exactly 2 ACT instrs ahead of attention
                    b0 = 64 * p
                    sl = slice(b0, b0 + 64)
                    nc.vector.tensor_scalar_max(ss8[sl, :], ss8[sl, :],
                                                EPS * EPS)
                    nc.scalar.activation(ln8[sl, :], ss8[sl, :], Ln)
                    nc.scalar.activation(rs8[sl, :], ln8[sl, :], Exp,
                                         scale=-0.5, bias=biasln[sl, :])
                    for a in (2 * p, 2 * p + 1):
                        nc.sync.dma_start(rsd[2 * a:2 * a + 2, :],
                                          rs8[32 * a:32 * a + 2, :])

                def norm_chunk(p, off, cw, rsbq, rsbk):
                    for (dst, a, src_m, rsb) in (
                            (qhat[p], 2 * p, p, rsbq),
                            (khat[p], 2 * p + 1, 2 + p, rsbk)):
                        bcast_row(rsd[2 * a][off:off + cw],
                                  rsb[0:64, off:off + cw], 64)
                        bcast_row(rsd[2 * a + 1][off:off + cw],
                                  rsb[64:128, off:off + cw], 64)
                        nc.vector.tensor_mul(dst[:, off:off + cw],
                                             qk_sb[src_m][:, off:off + cw],
                                             rsb[:, off:off + cw])

                # pair 0, chunk-major: attention can start after chunk 0.
                # q copies on ACT, k copies on DVE to balance the two queues.
                rsbq0 = RSB.tile([128, N], F32, tag="rsb", name="rsbq0")
                rsbk0 = RSB.tile([128, N], F32, tag="rsb", name="rsbk0")
                for (off, cw) in CHUNKS:
                    qkv_chunk(0, 0, off, cw, "dve", PSQ, PSS, "pq", "pss")
                    qkv_chunk(2, 1, off, cw, "dve", PSQ, PSS, "pq", "pss")
                rs_batched(0)
                for (off, cw) in CHUNKS:
                    norm_chunk(0, off, cw, rsbq0, rsbk0)

                # v^T via x^T @ w_v^T (PE work; overlaps the pair-0 chain)
                for jt in range(NJ):
                    pv = PSV.tile([128, 256], F32, tag="pv", name="pv")
                    for c in range(2):
                        nc.tensor.matmul(
                            pv[:, :],
                            mc(x_sb[c][:, jt * 128:(jt + 1) * 128]),
                            mc(wvT_sb[c][:, :]),
                            start=(c == 0), stop=(c == 1))
                    nc.vector.tensor_copy(
                        vT_sb[:, jt, :, :],
                        pv.rearrange("p (h d) -> p h d", h=4))


            # ---- phase 2+3: attention, scaling, output projection ----
            with tc.tile_pool(name="pssim", bufs=2, space="PSUM") as PSSIM, \
                 tc.tile_pool(name="pso", bufs=1, space="PSUM") as PSO:

                def attention_pair(hp, chunks):
                    for (off, cw) in chunks:
                        po = PSO.tile([128, 512], F32, tag="po", name="po")
                        po_o = PSO.tile([33, 512], F32, tag="po_o",
                                        name="po_o")

                        def sim_pair(jt, ps):
                            js = slice(jt * 128, (jt + 1) * 128)
                            nc.tensor.matmul(
                                ps[:, 0:cw],
                                mc(khat[hp][0:64, js]),
                                mc(qhat[hp][0:64, off:off + cw]),
                                start=True, stop=True, tile_position=(0, 0))
                            nc.tensor.matmul(
                                ps[:, 512:512 + cw],
                                mc(khat[hp][64:128, js]),
                                mc(qhat[hp][64:128, off:off + cw]),
                                start=True, stop=True, tile_position=(64, 0))

                        def ev_group(jt, eh0, eh1):
                            st, sp = (jt == 0), (jt == NJ - 1)
                            nc.tensor.matmul(
                                po[0:64, 0:cw],
                                mc(vT_sb[:, jt, 2 * hp, :]),
                                mc(eh0),
                                start=st, stop=sp, tile_position=(0, 0),
                                skip_group_check=True)
                            nc.tensor.matmul(
                                po[64:128, 0:cw],
                                mc(vT_sb[:, jt, 2 * hp + 1, :]),
                                mc(eh1),
                                start=st, stop=sp, tile_position=(0, 64),
                                skip_group_check=True)
                            nc.tensor.matmul(
                                po_o[0:1, 0:cw],
                                mc(ones8_sb[:, 8:9]),
                                mc(eh0),
                                start=st, stop=sp, tile_position=(0, 0),
                                skip_group_check=True)
                            nc.tensor.matmul(
                                po_o[32:33, 0:cw],
                                mc(ones8_sb[:, 8:9]),
                                mc(eh1),
                                start=st, stop=sp, tile_position=(0, 32),
                                skip_group_check=True)

                        # E@v trails 3 j's behind so the next chunk's
                        # first E@v (which waits the previous chunk's po
                        # drain) never blocks early sims on the in-order PE
                        pend = []
                        for jt in range(NJ):
                            ps = PSSIM.tile([128, 1024], F32, tag="ps",
                                            name="ps")
                            sim_pair(jt, ps)
                            e = ESB.tile([128, 1024], WD, tag="e",
                                         name="e")
                            ps3 = ps.rearrange("p (b c) -> p b c", b=2)
                            e3b = e.rearrange("p (b c) -> p b c", b=2)
                            nc.scalar.activation(e3b[:, :, 0:cw],
                                                 ps3[:, :, 0:cw], Exp)
                            pend.append((jt, e))
                            if len(pend) > 3:
                                j0, ee = pend.pop(0)
                                ev_group(j0, ee[:, 0:cw],
                                         ee[:, 512:512 + cw])
                        for (j0, ee) in pend:
                            ev_group(j0, ee[:, 0:cw], ee[:, 512:512 + cw])
                        # drain numerators + denominators (s rows at base 32h)
                        nc.vector.tensor_copy(numer[hp][:, off:off + cw],
                                              po[:, 0:cw])
                        for t in range(2):
                            h = 2 * hp + t
                            nc.vector.tensor_copy(
                                s8[32 * h:32 * h + 1, off:off + cw],
                                po_o[32 * t:32 * t + 1, 0:cw])

                def scale_pair(hp):
                    # 1/s for the two heads of this pair, broadcast, rescale.
                    # pair 0 uses the DVE divider (idle while pair-1 attention
                    # saturates ACT); pair 1 uses ACT in the idle tail.
                    b0 = 64 * hp
                    sl = slice(b0, b0 + 64)
                    if hp == 0:
                        nc.vector.reciprocal(rsden8[sl, :], s8[sl, :])
                    else:
                        nc.scalar.activation(lnd8[sl, :], s8[sl, :], Ln)
                        nc.scalar.activation(rsden8[sl, :], lnd8[sl, :], Exp,
                                             scale=-1.0)
                    for t in range(2):
                        h = 2 * hp + t
                        nc.sync.dma_start(rsdd[h:h + 1, :],
                                          rsden8[32 * h:32 * h + 1, :])
                    rsb = RSB.tile([128, N], F32, tag="rsb", name="rsb")
                    for (off, cw) in CHUNKS:
                        bcast_row(rsdd[2 * hp][off:off + cw],
                                  rsb[0:64, off:off + cw], 64)
                        bcast_row(rsdd[2 * hp + 1][off:off + cw],
                                  rsb[64:128, off:off + cw], 64)
                        nc.vector.tensor_mul(nsc[hp][:, off:off + cw],
                                             numer[hp][:, off:off + cw],
                                             rsb[:, off:off + cw])

                def outproj_pair(pr):
                    for m2 in range(2):
                        for (off, cw) in CHUNKS:
                            pf = PSF.tile([128, 512], F32, tag="pf",
                                          name="pf")
                            nc.tensor.matmul(
                                pf[:, 0:cw],
                                mc(woT_sb[pr][:, m2 * 128:(m2 + 1) * 128]),
                                mc(nsc[pr][:, off:off + cw]),
                                start=True, stop=True)
                            yt = YST.tile([128, 512], F32, tag="yt",
                                          name="yt")
                            if pr == 0:
                                nc.vector.tensor_scalar_add(
                                    yt[:, 0:cw], pf[:, 0:cw],
                                    bias_sb[m2][:, :])
                            else:
                                nc.vector.tensor_copy(yt[:, 0:cw],
                                                      pf[:, 0:cw])
                            nc.sync.dma_start(y[pr][m2][:, off:off + cw],
                                              yt[:, 0:cw])

                # hp0 chunk 0 first, then pair-1 QKV (DVE copies,
                # PSF psum slots, chunked rs on ACT) hidden inside the hp0
                # attention window, then the rest.
                # pair-1 QKV is spread one chunk at a time between hp0's
                # attention chunks so its PE matmuls never queue en masse
                # ahead of later sims on the in-order PE stream.
                rsbq1 = RSB.tile([128, N], F32, tag="rsb", name="rsbq1")
                rsbk1 = RSB.tile([128, N], F32, tag="rsb", name="rsbk1")
                for ci, (off, cw) in enumerate(CHUNKS):
                    attention_pair(0, [CHUNKS[ci]])
                    qkv_chunk(1, 2, off, cw, "dve", PSF, PSF, "pf", "pf")
                    qkv_chunk(3, 3, off, cw, "dve", PSF, PSF, "pf", "pf")
                    if ci == len(CHUNKS) - 1:
                        rs_batched(1)
                        for (off2, cw2) in CHUNKS:
                            norm_chunk(1, off2, cw2, rsbq1, rsbk1)
                # pair-0 1/s via the DVE divider, chunk by chunk between
                # hp1's attention chunks (DVE is idle there); outproj0 runs
                # in hp1's PE slack afterwards.
                rsb0 = RSB.tile([128, N], F32, tag="rsb", name="rsb0")
                for ci, ch in enumerate(CHUNKS):
                    attention_pair(1, [ch])
                    off, cw = ch
                    nc.vector.reciprocal(rsden8[0:64, off:off + cw],
                                         s8[0:64, off:off + cw])
                    for t in range(2):
                        nc.sync.dma_start(rsdd[t:t + 1, off:off + cw],
                                          rsden8[32 * t:32 * t + 1,
                                                 off:off + cw])
                    bcast_row(rsdd[0][off:off + cw],
                              rsb0[0:64, off:off + cw], 64)
                    bcast_row(rsdd[1][off:off + cw],
                              rsb0[64:128, off:off + cw], 64)
                    nc.vector.tensor_mul(nsc[0][:, off:off + cw],
                                         numer[0][:, off:off + cw],
                                         rsb0[:, off:off + cw])
                outproj_pair(0)
                # pair-1 scale + output projection pipelined per chunk in
                # the idle tail (chunked ln/exp shortens the last-chunk
                # critical chain)
                rsb1 = RSB.tile([128, N], F32, tag="rsb", name="rsb1")
                for (off, cw) in CHUNKS:
                    nc.scalar.activation(lnd8[64:128, off:off + cw],
                                         s8[64:128, off:off + cw], Ln)
                    nc.scalar.activation(rsden8[64:128, off:off + cw],
                                         lnd8[64:128, off:off + cw], Exp,
                                         scale=-1.0)
                    for t in (2, 3):
                        nc.sync.dma_start(rsdd[t:t + 1, off:off + cw],
                                          rsden8[32 * t:32 * t + 1,
                                                 off:off + cw])
                    bcast_row(rsdd[2][off:off + cw],
                              rsb1[0:64, off:off + cw], 64)
                    bcast_row(rsdd[3][off:off + cw],
                              rsb1[64:128, off:off + cw], 64)
                    nc.vector.tensor_mul(nsc[1][:, off:off + cw],
                                         numer[1][:, off:off + cw],
                                         rsb1[:, off:off + cw])
                    for m2 in range(2):
                        pf = PSF.tile([128, 512], F32, tag="pf", name="pf")
                        nc.tensor.matmul(
                            pf[:, 0:cw],
                            mc(woT_sb[1][:, m2 * 128:(m2 + 1) * 128]),
                            mc(nsc[1][:, off:off + cw]),
                            start=True, stop=True)
                        yt = YST.tile([128, 512], F32, tag="yt", name="yt")
                        nc.vector.tensor_copy(yt[:, 0:cw], pf[:, 0:cw])
                        nc.sync.dma_start(y[1][m2][:, off:off + cw],
                                          yt[:, 0:cw])

    nc.compile()
    return nc


def _get_program(wd_name=WD_NAME):
    if wd_name not in _CACHE:
        _CACHE[wd_name] = _build(wd_name)
    return _CACHE[wd_name]


def _np_wd(wd_name):
    if wd_name == "bf16":
        import ml_dtypes
        return np.dtype(ml_dtypes.bfloat16)
    return np.dtype(np.float32)


def make_in_maps(x, w_qkv, w_out, b_out, wd_name=WD_NAME):
    x = np.asarray(x, np.float32)
    w_qkv = np.asarray(w_qkv, np.float32)
    w_out = np.asarray(w_out, np.float32)
    b_out = np.asarray(b_out, np.float32)
    wd = _np_wd(wd_name)

    ones8 = np.zeros((128, 9), np.float32)
    ones8[:, 8] = 1.0
    for cc in range(8):
        lo = 64 * (cc % 2)
        ones8[lo:lo + 64, cc] = 1.0

    in_maps = []
    for core in range(8):
        b, half = core // 2, core % 2
        hsel = slice(256 * half, 256 * (half + 1))
        q_rows = np.arange(0, 512)[hsel]
        k_rows = 512 + q_rows
        v_rows = 1024 + q_rows
        wqk_h = np.ascontiguousarray(
            w_qkv[np.r_[q_rows, k_rows], :].T).reshape(2, 128, 512)
        wvT_h = np.ascontiguousarray(w_qkv[v_rows, :].T).reshape(2, 128, 256)
        woT_h = np.ascontiguousarray(w_out[:, hsel].T).reshape(2, 128, 256)
        bias_h = (b_out if half == 0 else np.zeros_like(b_out))
        in_maps.append({
            "x2": x[b].reshape(C, N).reshape(2, 128, N).astype(wd),
            "wqk": wqk_h.astype(wd),
            "wvT": wvT_h.astype(wd),
            "woT": woT_h.astype(wd),
            "bias": bias_h.reshape(2, 128, 1).astype(np.float32),
            "ones8": ones8.astype(wd),
        })
    return in_maps


def gather_output(results):
    outs = [r["y"].sum(axis=0).reshape(C, N) for r in results]
    return np.stack([
        (outs[2 * b] + outs[2 * b + 1]).reshape(C, H, W) for b in range(B)
    ]).astype(np.float32)


def run(in_maps, wd_name=WD_NAME, **kwargs):
    from concourse import bass_utils
    nc = _get_program(wd_name)
    return bass_utils.run_bass_kernel_spmd(nc, in_maps,
                                           core_ids=list(range(8)), **kwargs)


def kernel(x, w_qkv, w_out, b_out):
    in_maps = make_in_maps(x, w_qkv, w_out, b_out)
    res = run(in_maps)
    return gather_output(res.results)

